# revision 1
# baseline (speedup 1.0000x reference)
"""Causal self-attention with T5 relative-position bias, distributed over
8 NeuronCores (batch x head-group parallel).

Problem: x[2,2048,1024] @ w_qkv -> 16-head causal attention with a T5
bucketed relative-position bias added to the scores -> @ w_proj.

Sharding: core c handles batch b = c//4 and heads [4*(c%4), 4*(c%4)+4).
Each core computes a partial output projection (its heads' slice of the
c_proj contraction); the host sums the 4 partials per batch.

On-chip dataflow (per core, all matmuls via TensorE in float32r unless
noted):
  xT [C,T] (host pre-transposed) -> Q^T,K^T [256,T] and V [T,256]
  per head: scores^T[k,q] = (K^T)^T(slice) @ Q^T(slice)  (scale folded
  into wq host-side); exp on ScalarE (PSUM->bf16); multiply by
  host-precomputed exp(bias) diagonal-band table (bf16, also applies the
  causal mask via zeros); A@V in bf16 with a ones-column appended to V so
  the softmax denominator falls out of the same matmul; normalize with
  VectorE reciprocal + partition-broadcast; final projection in f32r.
"""
import math
from contextlib import ExitStack

import numpy as np
import ml_dtypes

import concourse.bass as bass
import concourse.bacc as bacc
import concourse.mybir as mybir
import concourse.tile as tile
from concourse.bass_utils import run_bass_kernel_spmd

# Problem constants (hardcoded per contract)
B, T, C, H = 2, 2048, 1024, 16
D = C // H                      # 64
NUM_BUCKETS, MAX_DISTANCE = 32, 2048
N_CORES = 8
HPC = 4                         # heads per core
KT_N = T // 128                 # 16 k-tiles
W_EXPB = 2560                   # diag table width; expb[i,m] = e(m-i-512)

F32 = mybir.dt.float32
F32R = mybir.dt.float32r
BF16 = mybir.dt.bfloat16
EXP = mybir.ActivationFunctionType.Exp


# ---------------------------------------------------------------- host math
def _bucket_causal(d):
    """T5 causal bucket for distances d>=0.

    Runs the same jnp ops as the reference on the default jax backend so
    that discrete bucket boundaries match the graded reference bit-exactly
    (the trn/axon backend rounds f32->int32 where numpy truncates).
    """
    import jax.numpy as jnp

    rp = jnp.asarray(np.asarray(d, dtype=np.int32))
    max_exact = NUM_BUCKETS // 2
    is_small = rp < max_exact
    rp_safe = jnp.maximum(rp, 1).astype(jnp.float32)
    large = max_exact + (
        jnp.log(rp_safe / max_exact)
        / math.log(MAX_DISTANCE / max_exact)
        * (NUM_BUCKETS - max_exact)
    ).astype(jnp.int32)
    large = jnp.minimum(large, NUM_BUCKETS - 1)
    return np.asarray(jnp.where(is_small, rp, large))


def _expb_tables(rel_table, h0):
    """[HPC, 128, W_EXPB] bf16: expb[lh][i, m] = exp(bias(d)) at d = m-i-512,
    zero for d < 0 (applies the causal mask)."""
    # vec[j] for j = m - i + 127, so d = j - 639
    j = np.arange(W_EXPB + 127)
    d = j - 639
    valid = d >= 0
    buckets = _bucket_causal(np.where(valid, d, 0))
    out = np.zeros((HPC, 128, W_EXPB), dtype=np.float32)
    i_idx = np.arange(128)[:, None]
    m_idx = np.arange(W_EXPB)[None, :]
    jj = m_idx - i_idx + 127
    for lh in range(HPC):
        vec = np.where(valid, np.exp(rel_table[buckets, h0 + lh]), 0.0).astype(
            np.float32
        )
        out[lh] = vec[jj]
    return out.astype(ml_dtypes.bfloat16)


def host_in_maps(x, w_qkv, w_proj, rel_table):
    """Build the 8 per-core input maps."""
    x = np.asarray(x, dtype=np.float32)
    w_qkv = np.asarray(w_qkv, dtype=np.float32)
    w_proj = np.asarray(w_proj, dtype=np.float32)
    rel_table = np.asarray(rel_table, dtype=np.float32)
    scale = 1.0 / math.sqrt(D)
    in_maps = []
    xT = [np.ascontiguousarray(x[b].T) for b in range(B)]
    for c in range(N_CORES):
        b, h0 = c // 4, 4 * (c % 4)
        cs = slice(64 * h0, 64 * h0 + 256)
        in_maps.append(
            {
                "xT": xT[b],
                "wq": np.ascontiguousarray(w_qkv[:, cs] * scale),
                "wk": np.ascontiguousarray(w_qkv[:, 1024 + 64 * h0 : 1024 + 64 * h0 + 256]),
                "wv": np.ascontiguousarray(w_qkv[:, 2048 + 64 * h0 : 2048 + 64 * h0 + 256]),
                "wp": np.ascontiguousarray(w_proj[cs, :]),
                "expb": _expb_tables(rel_table, h0),
            }
        )
    return in_maps


# ------------------------------------------------------------- bass program
def build_program():
    nc = bacc.Bacc("TRN2", target_bir_lowering=False, debug=False)
    XT = nc.dram_tensor("xT", [C, T], F32R, kind="ExternalInput")
    WQ = nc.dram_tensor("wq", [C, 256], F32R, kind="ExternalInput")
    WK = nc.dram_tensor("wk", [C, 256], F32R, kind="ExternalInput")
    WV = nc.dram_tensor("wv", [C, 256], F32R, kind="ExternalInput")
    WP = nc.dram_tensor("wp", [256, C], F32R, kind="ExternalInput")
    EXPB = nc.dram_tensor("expb", [HPC, 128, W_EXPB], BF16, kind="ExternalInput")
    OUT = nc.dram_tensor("out", [T, C], F32, kind="ExternalOutput")

    with tile.TileContext(nc) as tc, ExitStack() as ctx:
        persist = ctx.enter_context(tc.tile_pool(name="persist", bufs=1))
        work = ctx.enter_context(tc.tile_pool(name="work", bufs=1))

        # ---- persistent tiles
        QT = [persist.tile([128, T], F32R, tag=f"qt{g}", name=f"qt{g}") for g in range(2)]
        KT = [persist.tile([128, T], F32R, tag=f"kt{g}", name=f"kt{g}") for g in range(2)]
        V = [persist.tile([128, KT_N * 65], BF16, tag=f"v{lh}", name=f"v{lh}") for lh in range(HPC)]
        # per head-pair exp(bias) tables, side by side so one 3D AP covers both
        EB = [persist.tile([128, 2 * W_EXPB], BF16, tag=f"eb{g}", name=f"eb{g}") for g in range(2)]
        YN = [persist.tile([128, T], F32R, tag=f"yn{g}", name=f"yn{g}") for g in range(2)]
        WPS = [persist.tile([128, C], F32R, tag=f"wp{i}", name=f"wp{i}") for i in range(2)]

        # ===== interleaved QKV / attention phasing ============================
        # A1: QT0/KT0 + V(all heads) -> B1: attention pair 0
        # A2: QT1/KT1              -> B2: attention pair 1 -> C: projection
        xw = ctx.enter_context(tc.tile_pool(name="xw", bufs=1))
        xt_sb = xw.tile([128, 8 * T], F32R, tag="xt")
        wq_sb = xw.tile([128, 8 * 256], F32R, tag="wq")
        wk_sb = xw.tile([128, 8 * 256], F32R, tag="wk")
        wv_sb = xw.tile([128, 8 * 256], F32R, tag="wv")
        for ct in range(8):
            nc.sync.dma_start(
                xt_sb[:, T * ct : T * (ct + 1)], XT[128 * ct : 128 * (ct + 1), :]
            )
            nc.sync.dma_start(
                wq_sb[:, 256 * ct : 256 * (ct + 1)], WQ[128 * ct : 128 * (ct + 1), :]
            )
            nc.sync.dma_start(
                wk_sb[:, 256 * ct : 256 * (ct + 1)], WK[128 * ct : 128 * (ct + 1), :]
            )
        for ct in range(8):
            nc.sync.dma_start(
                wv_sb[:, 256 * ct : 256 * (ct + 1)], WV[128 * ct : 128 * (ct + 1), :]
            )
        for g in range(2):
            for s in range(2):
                nc.sync.dma_start(
                    EB[g][:, W_EXPB * s : W_EXPB * (s + 1)], EXPB[2 * g + s, :, :]
                )
        for i in range(2):
            nc.sync.dma_start(WPS[i][:], WP[128 * i : 128 * (i + 1), :])
        for lh in range(HPC):
            nc.vector.memset(V[lh][:], 1.0)

        def qkv_pair(g, ph):
            psA = ph.enter_context(tc.tile_pool(name=f"psA{g}", bufs=4, space="PSUM"))
            for w_sb, dst in ((wq_sb, QT[g]), (wk_sb, KT[g])):
                for n in range(4):
                    ps = psA.tile([128, 512], F32, tag="qkv", name="qkvps")
                    for ct in range(8):
                        nc.tensor.matmul(
                            ps[:],
                            w_sb[:, 256 * ct + 128 * g : 256 * ct + 128 * (g + 1)],
                            xt_sb[:, T * ct + 512 * n : T * ct + 512 * (n + 1)],
                            start=(ct == 0),
                            stop=(ct == 7),
                        )
                    nc.vector.tensor_copy(dst[:, 512 * n : 512 * (n + 1)], ps[:])

        def v_all(ph):
            psV = ph.enter_context(tc.tile_pool(name="psV", bufs=2, space="PSUM"))
            for tt in range(KT_N):
                ps = psV.tile([128, 256], F32, tag="vps", name="vps")
                for ct in range(8):
                    nc.tensor.matmul(
                        ps[:],
                        xt_sb[:, T * ct + 128 * tt : T * ct + 128 * (tt + 1)],
                        wv_sb[:, 256 * ct : 256 * (ct + 1)],
                        start=(ct == 0),
                        stop=(ct == 7),
                    )
                for lh in range(HPC):
                    nc.vector.tensor_copy(
                        V[lh][:, 65 * tt : 65 * tt + 64], ps[:, 64 * lh : 64 * (lh + 1)]
                    )

        def attention_pair(g, ph):
            psS = ph.enter_context(tc.tile_pool(name=f"psS{g}", bufs=1, space="PSUM"))
            psAV = ph.enter_context(tc.tile_pool(name=f"psAV{g}", bufs=1, space="PSUM"))
            eb3 = EB[g][:].rearrange("p (s w) -> p s w", s=2)
            for jc in range(4):       # 512-wide q chunk
                q0 = 512 * jc
                kt_max = (q0 + 511) // 128   # inclusive last k-tile
                # av holds both heads: h0 cols 0:512, h1 cols 512:1024
                av = psAV.tile([128, 1024], F32, tag="av", bufs=2, name="av")
                pend = None     # AV issued one k behind so PE never waits DVE
                for k in range(kt_max + 1):
                    k0 = 128 * k
                    # both heads' scores into one [128, 1024] psum tile
                    ps = psS.tile([128, 1024], F32, tag="s", bufs=2, name="sps")
                    for s in range(2):
                        nc.tensor.matmul(
                            ps[:, 512 * s : 512 * (s + 1)],
                            KT[g][64 * s : 64 * (s + 1), k0 : k0 + 128],
                            QT[g][64 * s : 64 * (s + 1), q0 : q0 + 512],
                            start=True,
                            stop=True,
                        )
                    if pend is not None:
                        _emit_av(g, av, *pend, kt_max)
                    # one exp + one in-place expb-multiply covering both heads;
                    # columns q < k0 are fully causal-masked -> skip them
                    lo2 = max(0, k0 - q0)
                    ps3 = ps[:].rearrange("p (s w) -> p s w", s=2)
                    att = work.tile([128, 1024], BF16, tag="att", bufs=3, name="att")
                    at3 = att[:].rearrange("p (s w) -> p s w", s=2)
                    nc.scalar.activation(at3[:, :, lo2:512], ps3[:, :, lo2:512], EXP)
                    m0 = q0 - k0 + 512
                    nc.vector.tensor_mul(
                        at3[:, :, lo2:512], at3[:, :, lo2:512],
                        eb3[:, :, m0 + lo2 : m0 + 512],
                    )
                    pend = (k, att, lo2)
                _emit_av(g, av, *pend, kt_max)
                # normalize: yn = av[0:64] * (1/av[64]) broadcast
                for s in range(2):
                    rcp = work.tile([1, 512], F32, tag="rcp", bufs=2, name="rcp")
                    nc.vector.reciprocal(rcp[:], av[64:65, 512 * s : 512 * (s + 1)])
                    bc = work.tile([64, 512], F32, tag="bc", bufs=2, name="bc")
                    nc.gpsimd.partition_broadcast(bc[:], rcp[:])
                    nc.vector.tensor_mul(
                        YN[g][64 * s : 64 * (s + 1), q0 : q0 + 512],
                        av[0:64, 512 * s : 512 * (s + 1)],
                        bc[:],
                    )

        def _emit_av(g, av, k, att, lo2, kt_max):
            for s in range(2):
                nc.tensor.matmul(
                    av[0:65, 512 * s + lo2 : 512 * (s + 1)],
                    V[2 * g + s][:, 65 * k : 65 * k + 65],
                    att[:, 512 * s + lo2 : 512 * (s + 1)],
                    start=(k == 0),
                    stop=(k == kt_max),
                )

        with ExitStack() as ph:
            qkv_pair(0, ph)
            v_all(ph)
        with ExitStack() as ph:
            attention_pair(0, ph)
        with ExitStack() as ph:
            qkv_pair(1, ph)
        with ExitStack() as ph:
            attention_pair(1, ph)

        # ======================= phase C: output projection ===================
        with ExitStack() as ph_c:
            psP = ph_c.enter_context(tc.tile_pool(name="psP", bufs=4, space="PSUM"))
            for tt in range(KT_N):
                osb = work.tile([128, C], F32, tag="osb", bufs=2)
                for co in range(2):
                    ps = psP.tile([128, 512], F32, tag="p")
                    for g in range(2):
                        nc.tensor.matmul(
                            ps[:],
                            YN[g][:, 128 * tt : 128 * (tt + 1)],
                            WPS[g][:, 512 * co : 512 * (co + 1)],
                            start=(g == 0),
                            stop=(g == 1),
                        )
                    nc.scalar.copy(osb[:, 512 * co : 512 * (co + 1)], ps[:])
                nc.sync.dma_start(OUT[128 * tt : 128 * (tt + 1), :], osb[:])

    nc.compile()
    return nc


_PROGRAM = None


def kernel(x, w_qkv, w_proj, rel_table):
    global _PROGRAM
    if _PROGRAM is None:
        _PROGRAM = build_program()
    in_maps = host_in_maps(x, w_qkv, w_proj, rel_table)
    res = run_bass_kernel_spmd(_PROGRAM, in_maps, core_ids=list(range(N_CORES)))
    out = np.zeros((B, T, C), dtype=np.float32)
    for c in range(N_CORES):
        out[c // 4] += res.results[c]["out"]
    return out



# revision 6
# speedup vs baseline: 69.9854x; 69.9854x over previous
"""Causal self-attention with T5 relative-position bias, distributed over
8 NeuronCores (batch x head-group parallel).

Problem: x[2,2048,1024] @ w_qkv -> 16-head causal attention with a T5
bucketed relative-position bias added to the scores -> @ w_proj.

Sharding: core c handles batch b = c//4 and heads [4*(c%4), 4*(c%4)+4).
Each core computes a partial output projection (its heads' slice of the
c_proj contraction); the host sums the 4 partials per batch.

On-chip dataflow (per core, all matmuls via TensorE in bf16 with f32
PSUM accumulation; activations/weights are pre-cast to bf16 host-side):
  xT [C,T] (host pre-transposed) -> Q^T,K^T [256,T] and V [T,256]
  per head: scores^T[k,q] = (K^T)^T(slice) @ Q^T(slice)  (scale folded
  into wq host-side); exp on ScalarE (PSUM->bf16); multiply by
  host-precomputed exp(bias) diagonal-band table (bf16, also applies the
  causal mask via zeros); A@V in bf16 with a ones-column appended to V so
  the softmax denominator falls out of the same matmul; normalize with
  VectorE reciprocal + partition-broadcast; final projection in bf16.
"""
import math
from contextlib import ExitStack

import numpy as np
import ml_dtypes

import concourse.bass as bass
import concourse.bacc as bacc
import concourse.mybir as mybir
import concourse.tile as tile
from concourse.bass_utils import run_bass_kernel_spmd

# Problem constants (hardcoded per contract)
B, T, C, H = 2, 2048, 1024, 16
D = C // H                      # 64
NUM_BUCKETS, MAX_DISTANCE = 32, 2048
N_CORES = 8
HPC = 4                         # heads per core
KT_N = T // 128                 # 16 k-tiles
W_EXPB = 2560                   # diag table width; expb[i,m] = e(m-i-512)

F32 = mybir.dt.float32
F32R = mybir.dt.float32r
BF16 = mybir.dt.bfloat16
EXP = mybir.ActivationFunctionType.Exp


# ---------------------------------------------------------------- host math
def _bucket_causal(d):
    """T5 causal bucket for distances d>=0.

    Runs the same jnp ops as the reference on the default jax backend so
    that discrete bucket boundaries match the graded reference bit-exactly
    (the trn/axon backend rounds f32->int32 where numpy truncates).
    """
    import jax.numpy as jnp

    rp = jnp.asarray(np.asarray(d, dtype=np.int32))
    max_exact = NUM_BUCKETS // 2
    is_small = rp < max_exact
    rp_safe = jnp.maximum(rp, 1).astype(jnp.float32)
    large = max_exact + (
        jnp.log(rp_safe / max_exact)
        / math.log(MAX_DISTANCE / max_exact)
        * (NUM_BUCKETS - max_exact)
    ).astype(jnp.int32)
    large = jnp.minimum(large, NUM_BUCKETS - 1)
    return np.asarray(jnp.where(is_small, rp, large))


def _expb_tables(rel_table, h0):
    """[HPC, 128, W_EXPB] bf16: expb[lh][i, m] = exp(bias(d)) at d = m-i-512,
    zero for d < 0 (applies the causal mask)."""
    # vec[j] for j = m - i + 127, so d = j - 639
    j = np.arange(W_EXPB + 127)
    d = j - 639
    valid = d >= 0
    buckets = _bucket_causal(np.where(valid, d, 0))
    out = np.zeros((HPC, 128, W_EXPB), dtype=np.float32)
    i_idx = np.arange(128)[:, None]
    m_idx = np.arange(W_EXPB)[None, :]
    jj = m_idx - i_idx + 127
    for lh in range(HPC):
        vec = np.where(valid, np.exp(rel_table[buckets, h0 + lh]), 0.0).astype(
            np.float32
        )
        out[lh] = vec[jj]
    return out.astype(ml_dtypes.bfloat16)


def host_in_maps(x, w_qkv, w_proj, rel_table):
    """Build the 8 per-core input maps (activations/weights pre-cast to
    bf16 host-side: halves the load DMA and runs every matmul at the PE's
    full bf16 rate; PSUM accumulation stays f32)."""
    x = np.asarray(x, dtype=np.float32)
    w_qkv = np.asarray(w_qkv, dtype=np.float32)
    w_proj = np.asarray(w_proj, dtype=np.float32)
    rel_table = np.asarray(rel_table, dtype=np.float32)
    scale = 1.0 / math.sqrt(D)
    bf = ml_dtypes.bfloat16
    in_maps = []
    xT = [np.ascontiguousarray(x[b].T).astype(bf) for b in range(B)]
    for c in range(N_CORES):
        b, h0 = c // 4, 4 * (c % 4)
        cs = slice(64 * h0, 64 * h0 + 256)
        in_maps.append(
            {
                "xT": xT[b],
                "wq": np.ascontiguousarray(w_qkv[:, cs] * scale).astype(bf),
                "wk": np.ascontiguousarray(
                    w_qkv[:, 1024 + 64 * h0 : 1024 + 64 * h0 + 256]
                ).astype(bf),
                "wv": np.ascontiguousarray(
                    w_qkv[:, 2048 + 64 * h0 : 2048 + 64 * h0 + 256]
                ).astype(bf),
                "wp": np.ascontiguousarray(w_proj[cs, :]).astype(bf),
                "expb": _expb_tables(rel_table, h0),
            }
        )
    return in_maps


# ------------------------------------------------------------- bass program
def build_program():
    nc = bacc.Bacc("TRN2", target_bir_lowering=False, debug=False)
    XT = nc.dram_tensor("xT", [C, T], BF16, kind="ExternalInput")
    WQ = nc.dram_tensor("wq", [C, 256], BF16, kind="ExternalInput")
    WK = nc.dram_tensor("wk", [C, 256], BF16, kind="ExternalInput")
    WV = nc.dram_tensor("wv", [C, 256], BF16, kind="ExternalInput")
    WP = nc.dram_tensor("wp", [256, C], BF16, kind="ExternalInput")
    EXPB = nc.dram_tensor("expb", [HPC, 128, W_EXPB], BF16, kind="ExternalInput")
    OUT = nc.dram_tensor("out", [T, C], F32, kind="ExternalOutput")

    with tile.TileContext(nc) as tc, ExitStack() as ctx:
        persist = ctx.enter_context(tc.tile_pool(name="persist", bufs=1))
        work = ctx.enter_context(tc.tile_pool(name="work", bufs=1))

        # ---- persistent tiles
        QT = [persist.tile([128, T], BF16, tag=f"qt{g}", name=f"qt{g}") for g in range(2)]
        KT = [persist.tile([128, T], BF16, tag=f"kt{g}", name=f"kt{g}") for g in range(2)]
        V = [persist.tile([128, KT_N * 65], BF16, tag=f"v{lh}", name=f"v{lh}") for lh in range(HPC)]
        # per head-pair exp(bias) tables, side by side so one 3D AP covers both
        EB = [persist.tile([128, 2 * W_EXPB], BF16, tag=f"eb{g}", name=f"eb{g}") for g in range(2)]
        YN = [persist.tile([128, T], BF16, tag=f"yn{g}", name=f"yn{g}") for g in range(2)]
        WPS = [persist.tile([128, C], BF16, tag=f"wp{i}", name=f"wp{i}") for i in range(2)]

        # ===== interleaved QKV / attention phasing ============================
        # A1: QT0/KT0 + V(all heads) -> B1: attention pair 0
        # A2: QT1/KT1              -> B2: attention pair 1 -> C: projection
        xw = ctx.enter_context(tc.tile_pool(name="xw", bufs=1))
        xt_sb = xw.tile([128, 8 * T], BF16, tag="xt")
        wq_sb = xw.tile([128, 8 * 256], BF16, tag="wq")
        wk_sb = xw.tile([128, 8 * 256], BF16, tag="wk")
        wv_sb = xw.tile([128, 8 * 256], BF16, tag="wv")
        for ct in range(8):
            nc.sync.dma_start(
                xt_sb[:, T * ct : T * (ct + 1)], XT[128 * ct : 128 * (ct + 1), :]
            )
            nc.sync.dma_start(
                wq_sb[:, 256 * ct : 256 * (ct + 1)], WQ[128 * ct : 128 * (ct + 1), :]
            )
            nc.sync.dma_start(
                wk_sb[:, 256 * ct : 256 * (ct + 1)], WK[128 * ct : 128 * (ct + 1), :]
            )
        for ct in range(8):
            nc.sync.dma_start(
                wv_sb[:, 256 * ct : 256 * (ct + 1)], WV[128 * ct : 128 * (ct + 1), :]
            )
        for g in range(2):
            for s in range(2):
                nc.sync.dma_start(
                    EB[g][:, W_EXPB * s : W_EXPB * (s + 1)], EXPB[2 * g + s, :, :]
                )
        for i in range(2):
            nc.sync.dma_start(WPS[i][:], WP[128 * i : 128 * (i + 1), :])
        for lh in range(HPC):
            nc.vector.memset(V[lh][:], 1.0)

        def qkv_pair(g, ph):
            psA = ph.enter_context(tc.tile_pool(name=f"psA{g}", bufs=4, space="PSUM"))
            for w_sb, dst in ((wq_sb, QT[g]), (wk_sb, KT[g])):
                for n in range(4):
                    ps = psA.tile([128, 512], F32, tag="qkv", name="qkvps")
                    for ct in range(8):
                        nc.tensor.matmul(
                            ps[:],
                            w_sb[:, 256 * ct + 128 * g : 256 * ct + 128 * (g + 1)],
                            xt_sb[:, T * ct + 512 * n : T * ct + 512 * (n + 1)],
                            start=(ct == 0),
                            stop=(ct == 7),
                        )
                    nc.vector.tensor_copy(dst[:, 512 * n : 512 * (n + 1)], ps[:])

        def v_all(ph):
            psV = ph.enter_context(tc.tile_pool(name="psV", bufs=2, space="PSUM"))
            for tt in range(KT_N):
                ps = psV.tile([128, 256], F32, tag="vps", name="vps")
                for ct in range(8):
                    nc.tensor.matmul(
                        ps[:],
                        xt_sb[:, T * ct + 128 * tt : T * ct + 128 * (tt + 1)],
                        wv_sb[:, 256 * ct : 256 * (ct + 1)],
                        start=(ct == 0),
                        stop=(ct == 7),
                    )
                for lh in range(HPC):
                    nc.vector.tensor_copy(
                        V[lh][:, 65 * tt : 65 * tt + 64], ps[:, 64 * lh : 64 * (lh + 1)]
                    )

        def attention_pair(g, ph):
            psS = ph.enter_context(tc.tile_pool(name=f"psS{g}", bufs=1, space="PSUM"))
            psAV = ph.enter_context(tc.tile_pool(name=f"psAV{g}", bufs=1, space="PSUM"))
            eb3 = EB[g][:].rearrange("p (s w) -> p s w", s=2)
            for jc in range(4):       # 512-wide q chunk
                q0 = 512 * jc
                kt_max = (q0 + 511) // 128   # inclusive last k-tile
                # av holds both heads: h0 cols 0:512, h1 cols 512:1024
                av = psAV.tile([128, 1024], F32, tag="av", bufs=2, name="av")
                pend = None     # AV issued one k behind so PE never waits DVE
                for k in range(kt_max + 1):
                    k0 = 128 * k
                    # both heads' scores into one [128, 1024] psum tile
                    ps = psS.tile([128, 1024], F32, tag="s", bufs=2, name="sps")
                    for s in range(2):
                        nc.tensor.matmul(
                            ps[:, 512 * s : 512 * (s + 1)],
                            KT[g][64 * s : 64 * (s + 1), k0 : k0 + 128],
                            QT[g][64 * s : 64 * (s + 1), q0 : q0 + 512],
                            start=True,
                            stop=True,
                        )
                    if pend is not None:
                        _emit_av(g, av, *pend, kt_max)
                    # one exp + one in-place expb-multiply covering both heads;
                    # columns q < k0 are fully causal-masked -> skip them
                    lo2 = max(0, k0 - q0)
                    ps3 = ps[:].rearrange("p (s w) -> p s w", s=2)
                    att = work.tile([128, 1024], BF16, tag="att", bufs=3, name="att")
                    at3 = att[:].rearrange("p (s w) -> p s w", s=2)
                    nc.scalar.activation(at3[:, :, lo2:512], ps3[:, :, lo2:512], EXP)
                    m0 = q0 - k0 + 512
                    nc.vector.tensor_mul(
                        at3[:, :, lo2:512], at3[:, :, lo2:512],
                        eb3[:, :, m0 + lo2 : m0 + 512],
                    )
                    pend = (k, att, lo2)
                _emit_av(g, av, *pend, kt_max)
                # normalize: yn = av[0:64] * (1/av[64]) broadcast
                for s in range(2):
                    rcp = work.tile([1, 512], F32, tag="rcp", bufs=2, name="rcp")
                    nc.vector.reciprocal(rcp[:], av[64:65, 512 * s : 512 * (s + 1)])
                    bc = work.tile([64, 512], F32, tag="bc", bufs=2, name="bc")
                    nc.gpsimd.partition_broadcast(bc[:], rcp[:])
                    nc.vector.tensor_mul(
                        YN[g][64 * s : 64 * (s + 1), q0 : q0 + 512],
                        av[0:64, 512 * s : 512 * (s + 1)],
                        bc[:],
                    )

        def _emit_av(g, av, k, att, lo2, kt_max):
            for s in range(2):
                nc.tensor.matmul(
                    av[0:65, 512 * s + lo2 : 512 * (s + 1)],
                    V[2 * g + s][:, 65 * k : 65 * k + 65],
                    att[:, 512 * s + lo2 : 512 * (s + 1)],
                    start=(k == 0),
                    stop=(k == kt_max),
                )

        with ExitStack() as ph:
            qkv_pair(0, ph)
            v_all(ph)
        with ExitStack() as ph:
            attention_pair(0, ph)
        with ExitStack() as ph:
            qkv_pair(1, ph)
        with ExitStack() as ph:
            attention_pair(1, ph)

        # ======================= phase C: output projection ===================
        with ExitStack() as ph_c:
            psP = ph_c.enter_context(tc.tile_pool(name="psP", bufs=4, space="PSUM"))
            for tt in range(KT_N):
                osb = work.tile([128, C], F32, tag="osb", bufs=2)
                for co in range(2):
                    ps = psP.tile([128, 512], F32, tag="p")
                    for g in range(2):
                        nc.tensor.matmul(
                            ps[:],
                            YN[g][:, 128 * tt : 128 * (tt + 1)],
                            WPS[g][:, 512 * co : 512 * (co + 1)],
                            start=(g == 0),
                            stop=(g == 1),
                        )
                    nc.scalar.copy(osb[:, 512 * co : 512 * (co + 1)], ps[:])
                nc.sync.dma_start(OUT[128 * tt : 128 * (tt + 1), :], osb[:])

    nc.compile()
    return nc


_PROGRAM = None


def kernel(x, w_qkv, w_proj, rel_table):
    global _PROGRAM
    if _PROGRAM is None:
        _PROGRAM = build_program()
    in_maps = host_in_maps(x, w_qkv, w_proj, rel_table)
    res = run_bass_kernel_spmd(_PROGRAM, in_maps, core_ids=list(range(N_CORES)))
    out = np.zeros((B, T, C), dtype=np.float32)
    for c in range(N_CORES):
        out[c // 4] += res.results[c]["out"]
    return out



# revision 16
# speedup vs baseline: 169.8275x; 2.4266x over previous
"""Causal self-attention with T5 relative-position bias, distributed over
8 NeuronCores (batch x head-group parallel).

Problem: x[2,2048,1024] @ w_qkv -> 16-head causal attention with a T5
bucketed relative-position bias added to the scores -> @ w_proj.

Sharding: core c handles batch b = c//4 and heads [4*(c%4), 4*(c%4)+4).
Each core computes a partial output projection (its heads' slice of the
c_proj contraction); the host sums the 4 partials per batch.

On-chip dataflow (per core, all matmuls via TensorE in bf16 with f32
PSUM accumulation; activations/weights are pre-cast to bf16 host-side):
  xT [C,T] (host pre-transposed) -> Q^T,K^T [256,T] and V [T,256]
  per head: scores^T[k,q] = (K^T)^T(slice) @ Q^T(slice)  (scale folded
  into wq host-side); exp on ScalarE (PSUM->bf16); multiply by
  host-precomputed exp(bias) diagonal-band table (bf16, also applies the
  causal mask via zeros); A@V in bf16 with a ones-column appended to V so
  the softmax denominator falls out of the same matmul; normalize with
  VectorE reciprocal + partition-broadcast; final projection in bf16.
"""
import math
from contextlib import ExitStack

import numpy as np
import ml_dtypes

import concourse.bass as bass
import concourse.bacc as bacc
import concourse.mybir as mybir
import concourse.tile as tile
from concourse.bass_utils import run_bass_kernel_spmd

# Problem constants (hardcoded per contract)
B, T, C, H = 2, 2048, 1024, 16
D = C // H                      # 64
NUM_BUCKETS, MAX_DISTANCE = 32, 2048
N_CORES = 8
HPC = 4                         # heads per core
KT_N = T // 128                 # 16 k-tiles
W_EXPB = 2560                   # diag table width; expb[i,m] = e(m-i-512)
# packed input blob: xT | wq | wk | wv | wp | expb (bf16 elements)
BLOB_ELEMS = C * T + 3 * (C * 256) + 256 * C + HPC * 128 * W_EXPB

F32 = mybir.dt.float32
F32R = mybir.dt.float32r
BF16 = mybir.dt.bfloat16
EXP = mybir.ActivationFunctionType.Exp


# ---------------------------------------------------------------- host math
def _bucket_causal(d):
    """T5 causal bucket for distances d>=0.

    Runs the same jnp ops as the reference on the default jax backend so
    that discrete bucket boundaries match the graded reference bit-exactly
    (the trn/axon backend rounds f32->int32 where numpy truncates).
    """
    import jax.numpy as jnp

    rp = jnp.asarray(np.asarray(d, dtype=np.int32))
    max_exact = NUM_BUCKETS // 2
    is_small = rp < max_exact
    rp_safe = jnp.maximum(rp, 1).astype(jnp.float32)
    large = max_exact + (
        jnp.log(rp_safe / max_exact)
        / math.log(MAX_DISTANCE / max_exact)
        * (NUM_BUCKETS - max_exact)
    ).astype(jnp.int32)
    large = jnp.minimum(large, NUM_BUCKETS - 1)
    return np.asarray(jnp.where(is_small, rp, large))


def _expb_tables(rel_table, h0):
    """[HPC, 128, W_EXPB] bf16: expb[lh][i, m] = exp(bias(d)) at d = m-i-512,
    zero for d < 0 (applies the causal mask)."""
    # vec[j] for j = m - i + 127, so d = j - 639
    j = np.arange(W_EXPB + 127)
    d = j - 639
    valid = d >= 0
    buckets = _bucket_causal(np.where(valid, d, 0))
    out = np.zeros((HPC, 128, W_EXPB), dtype=np.float32)
    i_idx = np.arange(128)[:, None]
    m_idx = np.arange(W_EXPB)[None, :]
    jj = m_idx - i_idx + 127
    for lh in range(HPC):
        vec = np.where(valid, np.exp(rel_table[buckets, h0 + lh]), 0.0).astype(
            np.float32
        )
        out[lh] = vec[jj]
    return out.astype(ml_dtypes.bfloat16)


def host_in_maps(x, w_qkv, w_proj, rel_table):
    """Build the 8 per-core input maps (activations/weights pre-cast to
    bf16 host-side: halves the load DMA and runs every matmul at the PE's
    full bf16 rate; PSUM accumulation stays f32).

    All sections are packed into ONE flat bf16 blob per core: a single
    ExternalInput means a single sharded arg to marshal per dispatch,
    which is what the per-execution dispatch cost scales with."""
    x = np.asarray(x, dtype=np.float32)
    w_qkv = np.asarray(w_qkv, dtype=np.float32)
    w_proj = np.asarray(w_proj, dtype=np.float32)
    rel_table = np.asarray(rel_table, dtype=np.float32)
    scale = 1.0 / math.sqrt(D)
    bf = ml_dtypes.bfloat16
    in_maps = []
    xT = [np.ascontiguousarray(x[b].T).astype(bf) for b in range(B)]
    for c in range(N_CORES):
        b, h0 = c // 4, 4 * (c % 4)
        cs = slice(64 * h0, 64 * h0 + 256)
        sections = [
            xT[b],
            np.ascontiguousarray(w_qkv[:, cs] * scale).astype(bf),
            np.ascontiguousarray(
                w_qkv[:, 1024 + 64 * h0 : 1024 + 64 * h0 + 256]
            ).astype(bf),
            np.ascontiguousarray(
                w_qkv[:, 2048 + 64 * h0 : 2048 + 64 * h0 + 256]
            ).astype(bf),
            np.ascontiguousarray(w_proj[cs, :]).astype(bf),
            _expb_tables(rel_table, h0),
        ]
        in_maps.append(
            {"blob": np.concatenate([s.ravel() for s in sections])}
        )
    return in_maps


# ------------------------------------------------------------- bass program
def build_program(repeat=1):
    """repeat>1 unrolls the whole kernel body N times in one NEFF — used
    only by benchmarking to separate device time from dispatch overhead."""
    nc = bacc.Bacc("TRN2", target_bir_lowering=False, debug=False)
    BLOB = nc.dram_tensor("blob", [BLOB_ELEMS], BF16, kind="ExternalInput")
    off = 0

    def _sec(shape):
        nonlocal off
        n = int(np.prod(shape))
        ap = BLOB[off : off + n]
        if len(shape) == 2:
            ap = ap.rearrange("(a b) -> a b", a=shape[0])
        elif len(shape) == 3:
            ap = ap.rearrange("(a b c) -> a b c", a=shape[0], b=shape[1])
        off += n
        return ap

    XT = _sec([C, T])
    WQ = _sec([C, 256])
    WK = _sec([C, 256])
    WV = _sec([C, 256])
    WP = _sec([256, C])
    EXPB = _sec([HPC, 128, W_EXPB])
    OUT = nc.dram_tensor("out", [T, C], BF16, kind="ExternalOutput")

    with tile.TileContext(nc) as tc:
        for _ in range(repeat):
            _build_body(nc, tc, XT, WQ, WK, WV, WP, EXPB, OUT)

    nc.compile()
    return nc


def _build_body(nc, tc, XT, WQ, WK, WV, WP, EXPB, OUT):
    with ExitStack() as ctx:
        persist = ctx.enter_context(tc.tile_pool(name="persist", bufs=1))
        work = ctx.enter_context(tc.tile_pool(name="work", bufs=1))

        # ---- persistent tiles
        QT = [persist.tile([128, T], BF16, tag=f"qt{g}", name=f"qt{g}") for g in range(2)]
        KT = [persist.tile([128, T], BF16, tag=f"kt{g}", name=f"kt{g}") for g in range(2)]
        V = [persist.tile([128, KT_N * 65], BF16, tag=f"v{lh}", name=f"v{lh}") for lh in range(HPC)]
        # per head-pair exp(bias) tables, side by side so one 3D AP covers both
        EB = [persist.tile([128, 2 * W_EXPB], BF16, tag=f"eb{g}", name=f"eb{g}") for g in range(2)]
        YN = [persist.tile([128, T], BF16, tag=f"yn{g}", name=f"yn{g}") for g in range(2)]
        WPS = [persist.tile([128, C], BF16, tag=f"wp{i}", name=f"wp{i}") for i in range(2)]

        # ===== interleaved QKV / attention phasing ============================
        # A1: QT0/KT0 + V(all heads) -> B1: attention pair 0
        # A2: QT1/KT1              -> B2: attention pair 1 -> C: projection
        xw = ctx.enter_context(tc.tile_pool(name="xw", bufs=1))
        xt_sb = xw.tile([128, 8 * T], BF16, tag="xt")
        wq_sb = xw.tile([128, 8 * 256], BF16, tag="wq")
        wk_sb = xw.tile([128, 8 * 256], BF16, tag="wk")
        wv_sb = xw.tile([128, 8 * 256], BF16, tag="wv")
        for ct in range(8):
            nc.sync.dma_start(
                xt_sb[:, T * ct : T * (ct + 1)], XT[128 * ct : 128 * (ct + 1), :]
            )
            nc.sync.dma_start(
                wq_sb[:, 256 * ct : 256 * (ct + 1)], WQ[128 * ct : 128 * (ct + 1), :]
            )
            nc.sync.dma_start(
                wk_sb[:, 256 * ct : 256 * (ct + 1)], WK[128 * ct : 128 * (ct + 1), :]
            )
        for ct in range(8):
            nc.sync.dma_start(
                wv_sb[:, 256 * ct : 256 * (ct + 1)], WV[128 * ct : 128 * (ct + 1), :]
            )
        for g in range(2):
            for s in range(2):
                nc.sync.dma_start(
                    EB[g][:, W_EXPB * s : W_EXPB * (s + 1)], EXPB[2 * g + s, :, :]
                )
        for i in range(2):
            nc.sync.dma_start(WPS[i][:], WP[128 * i : 128 * (i + 1), :])
        for lh in range(HPC):
            nc.vector.memset(V[lh][:], 1.0)

        def qkv_pair(g, ph):
            psA = ph.enter_context(tc.tile_pool(name=f"psA{g}", bufs=4, space="PSUM"))
            for w_sb, dst in ((wq_sb, QT[g]), (wk_sb, KT[g])):
                for n in range(4):
                    ps = psA.tile([128, 512], F32, tag="qkv", name="qkvps")
                    for ct in range(8):
                        nc.tensor.matmul(
                            ps[:],
                            w_sb[:, 256 * ct + 128 * g : 256 * ct + 128 * (g + 1)],
                            xt_sb[:, T * ct + 512 * n : T * ct + 512 * (n + 1)],
                            start=(ct == 0),
                            stop=(ct == 7),
                        )
                    nc.vector.tensor_copy(dst[:, 512 * n : 512 * (n + 1)], ps[:])

        def v_all(ph):
            psV = ph.enter_context(tc.tile_pool(name="psV", bufs=2, space="PSUM"))
            for tt in range(KT_N):
                ps = psV.tile([128, 256], F32, tag="vps", name="vps")
                for ct in range(8):
                    nc.tensor.matmul(
                        ps[:],
                        xt_sb[:, T * ct + 128 * tt : T * ct + 128 * (tt + 1)],
                        wv_sb[:, 256 * ct : 256 * (ct + 1)],
                        start=(ct == 0),
                        stop=(ct == 7),
                    )
                for lh in range(HPC):
                    nc.vector.tensor_copy(
                        V[lh][:, 65 * tt : 65 * tt + 64], ps[:, 64 * lh : 64 * (lh + 1)]
                    )

        def attention_pair(g, ph, fuse_proj=False):
            psS = ph.enter_context(tc.tile_pool(name=f"psS{g}", bufs=1, space="PSUM"))
            psAV = ph.enter_context(tc.tile_pool(name=f"psAV{g}", bufs=1, space="PSUM"))
            # when fusing the output projection into this pair, av drops to a
            # single buffer so psP fits in the remaining two PSUM banks
            av_bufs = 1 if fuse_proj else 2
            psP = (
                ph.enter_context(tc.tile_pool(name="psP", bufs=1, space="PSUM"))
                if fuse_proj
                else None
            )
            eb3 = EB[g][:].rearrange("p (s w) -> p s w", s=2)
            for jc in range(4):       # 512-wide q chunk
                q0 = 512 * jc
                kt_max = (q0 + 511) // 128   # inclusive last k-tile
                # av holds both heads: h0 cols 0:512, h1 cols 512:1024
                av = psAV.tile([128, 1024], F32, tag="av", bufs=av_bufs, name="av")
                pend = None     # AV issued one k behind so PE never waits DVE
                for k in range(kt_max + 1):
                    k0 = 128 * k
                    # both heads' scores into one [128, 1024] psum tile
                    ps = psS.tile([128, 1024], F32, tag="s", bufs=2, name="sps")
                    for s in range(2):
                        nc.tensor.matmul(
                            ps[:, 512 * s : 512 * (s + 1)],
                            KT[g][64 * s : 64 * (s + 1), k0 : k0 + 128],
                            QT[g][64 * s : 64 * (s + 1), q0 : q0 + 512],
                            start=True,
                            stop=True,
                        )
                    if pend is not None:
                        _emit_av(g, av, *pend, kt_max)
                    # one exp + one in-place expb-multiply covering both heads;
                    # columns q < k0 are fully causal-masked -> skip them
                    lo2 = max(0, k0 - q0)
                    ps3 = ps[:].rearrange("p (s w) -> p s w", s=2)
                    att = work.tile([128, 1024], BF16, tag="att", bufs=3, name="att")
                    at3 = att[:].rearrange("p (s w) -> p s w", s=2)
                    nc.scalar.activation(at3[:, :, lo2:512], ps3[:, :, lo2:512], EXP)
                    m0 = q0 - k0 + 512
                    nc.vector.tensor_mul(
                        at3[:, :, lo2:512], at3[:, :, lo2:512],
                        eb3[:, :, m0 + lo2 : m0 + 512],
                    )
                    pend = (k, att, lo2)
                _emit_av(g, av, *pend, kt_max)
                # normalize: yn = av[0:64] * (1/av[64]) broadcast
                for s in range(2):
                    rcp = work.tile([1, 512], F32, tag="rcp", bufs=2, name="rcp")
                    nc.vector.reciprocal(rcp[:], av[64:65, 512 * s : 512 * (s + 1)])
                    bc = work.tile([64, 512], F32, tag="bc", bufs=2, name="bc")
                    nc.gpsimd.partition_broadcast(bc[:], rcp[:])
                    nc.vector.tensor_mul(
                        YN[g][64 * s : 64 * (s + 1), q0 : q0 + 512],
                        av[0:64, 512 * s : 512 * (s + 1)],
                        bc[:],
                    )
                if fuse_proj:
                    # both YN groups are now final for columns q0:q0+512 ->
                    # project + write back these four 128-row output tiles
                    # while later q-chunks are still in flight
                    proj_tiles(psP, 4 * jc, 4 * jc + 4)

        def _emit_av(g, av, k, att, lo2, kt_max):
            for s in range(2):
                nc.tensor.matmul(
                    av[0:65, 512 * s + lo2 : 512 * (s + 1)],
                    V[2 * g + s][:, 65 * k : 65 * k + 65],
                    att[:, 512 * s + lo2 : 512 * (s + 1)],
                    start=(k == 0),
                    stop=(k == kt_max),
                )

        def proj_tiles(psP, t0, t1):
            for tt in range(t0, t1):
                osb = work.tile([128, C], BF16, tag="osb", bufs=2)
                for co in range(2):
                    ps = psP.tile([128, 512], F32, tag="p", bufs=2)
                    for g in range(2):
                        nc.tensor.matmul(
                            ps[:],
                            YN[g][:, 128 * tt : 128 * (tt + 1)],
                            WPS[g][:, 512 * co : 512 * (co + 1)],
                            start=(g == 0),
                            stop=(g == 1),
                        )
                    nc.vector.tensor_copy(osb[:, 512 * co : 512 * (co + 1)], ps[:])
                nc.sync.dma_start(OUT[128 * tt : 128 * (tt + 1), :], osb[:])

        with ExitStack() as ph:
            qkv_pair(0, ph)
            v_all(ph)
        with ExitStack() as ph:
            attention_pair(0, ph)
        with ExitStack() as ph:
            qkv_pair(1, ph)
        with ExitStack() as ph:
            attention_pair(1, ph, fuse_proj=True)


_PROGRAM = None


def kernel(x, w_qkv, w_proj, rel_table):
    global _PROGRAM
    if _PROGRAM is None:
        _PROGRAM = build_program()
    in_maps = host_in_maps(x, w_qkv, w_proj, rel_table)
    res = run_bass_kernel_spmd(_PROGRAM, in_maps, core_ids=list(range(N_CORES)))
    out = np.zeros((B, T, C), dtype=np.float32)
    for c in range(N_CORES):
        out[c // 4] += np.asarray(res.results[c]["out"], dtype=np.float32)
    return out

